# revision 16
# baseline (speedup 1.0000x reference)
"""Cost-sensitive cross-entropy loss on 8 TRN2 NeuronCores (Bass/Tile).

Data-parallel over the batch: each core streams its 8192x1000 logit shard;
per [128,1000] tile it computes row max (reduce_max), argmax via an
equality-mask * reverse-iota reduce_max, the softmax denominator via the
scalar engine's Exp with fused accumulation, and the target logit via a
per-tile indirect-DMA gather (one offset per partition).  Each sample then
contributes an ENC-encoded one-hot payload (count and value packed in one
f32) which is scatter-added into per-core HBM bins keyed by
(target*16 + predicted>>6).  dma_scatter_add does not serialize
read-modify-writes of the same 256B row within a call, so same-row
duplicates inside each 128-record tile are merged on the tensor engine
(equality-matrix matmul), non-first duplicates are routed to a garbage
row, and the per-tile calls form two WAW-serialized chains into separate
buffers that are then locally summed.  Phase 2 is software-pipelined one
chunk behind phase 1 so dedup/scatter overlap the streaming.  A
ReduceScatter combines bins across cores (class-sharded); each core
decodes counts/values, row-normalizes against the cost matrix and emits
[128] partial loss sums; the host sums them and applies -beta/B.

Self-contained: hardcodes B=65536, C=1000, beta=3.0, 8 cores.
"""
import sys
sys.path.insert(0, '/opt/trn_rl_repo')
import numpy as np
import concourse.bass as bass
import concourse.bacc as bacc
import concourse.mybir as mybir
from concourse import tile
from concourse.masks import make_identity
from concourse.bass_utils import run_bass_kernel_spmd

P = 128
C = 1000
B = 65536
N_CORES = 8
R = B // N_CORES
SUB = 16          # 64-wide sub-rows per class row (16*64 = 1024 >= C)
CW = 64           # scatter row width (f32) = 256B
ENC = 4096.0      # count encoding scale
SINGLE_PACKET = False
BETA = 3.0

F32 = mybir.dt.float32
I32 = mybir.dt.int32
I16 = mybir.dt.int16
U32 = mybir.dt.uint32
AF = mybir.ActivationFunctionType
ALU = mybir.AluOpType
AX = mybir.AxisListType


def build_kernel(R, n_cores, my_rank=None, dma_bufs=6, debug=False):
    NT = R // P
    NROW = C * SUB
    SR = NROW // n_cores
    Tsh = C // n_cores

    nc = bacc.Bacc(None, target_bir_lowering=False)
    xs = nc.dram_tensor("xs", [R, C], F32, kind="ExternalInput")
    tg = nc.dram_tensor("tg", [R], I32, kind="ExternalInput")
    cmrow = nc.dram_tensor("cmrow", [Tsh, C], F32, kind="ExternalInput")
    out = nc.dram_tensor("partial", [P, 1], F32, kind="ExternalOutput")

    bins_a = nc.dram_tensor("bins_a", [NROW + 1, CW], F32)
    bins_b = nc.dram_tensor("bins_b", [NROW + 1, CW], F32)
    rs_out = nc.dram_tensor("rs_out", [SR, CW], F32)

    xs_flat = xs[:].rearrange("a b -> (a b)")[:, None]
    za = bins_a[0:NROW, :].rearrange("(p n) c -> p (n c)", p=P)
    zb = bins_b[0:NROW, :].rearrange("(p n) c -> p (n c)", p=P)
    ZW = za.shape[1]
    H = ZW // 2

    HT = max(1, NT // 8)      # chunk size for pipelined phase 2
    NCH = NT // HT

    with tile.TileContext(nc) as tc:
        with (
            tc.tile_pool(name="xp", bufs=dma_bufs) as xp,
            tc.tile_pool(name="sp", bufs=2) as sp,
            tc.tile_pool(name="pp", bufs=1) as pp,
            tc.tile_pool(name="ps", bufs=2, space="PSUM") as psp,
        ):
            m8_all = pp.tile([P, NT * 8], F32)
            p8_all = pp.tile([P, NT * 8], U32)
            s_all = pp.tile([P, NT], F32)

            # targets + gather offsets
            t_sb = pp.tile([P, NT], I32)
            nc.sync.dma_start(out=t_sb[:], in_=tg[:].rearrange("(n p) -> p n", p=P))
            io_r = pp.tile([P, NT], I32)
            nc.gpsimd.iota(io_r[:], pattern=[[1, NT]], base=0, channel_multiplier=0)
            io_pc = pp.tile([P, 1], I32)
            nc.gpsimd.iota(io_pc[:], pattern=[[1, 1]], base=0, channel_multiplier=C)
            off = pp.tile([P, NT], I32)
            nc.vector.tensor_scalar(off[:], io_r[:], P * C, None, op0=ALU.mult)
            nc.vector.tensor_tensor(off[:], off[:], t_sb[:], op=ALU.add)
            nc.vector.tensor_tensor(
                off[:], off[:], io_pc[:].to_broadcast([P, NT]), op=ALU.add
            )
            x_t = pp.tile([P, NT], F32)

            logs = pp.tile([P, NT], F32)
            venc = pp.tile([P, NT], F32)
            pu = pp.tile([P, NT], U32)
            hi = pp.tile([P, NT], U32)
            hi_i = pp.tile([P, NT], I32)
            c6 = pp.tile([P, NT], U32)
            c6f = pp.tile([P, NT], F32)
            kp = pp.tile([P, NT], I32)
            kpf = pp.tile([P, NT], F32)
            keepm = pp.tile([P, NT], F32)
            payload = pp.tile([P, NT * CW], F32)
            pl3 = payload[:].rearrange("p (r c) -> p r c", c=CW)
            kadj = pp.tile([P, NT], F32)
            pay2 = pp.tile([P, NT * CW], F32)
            p23 = pay2[:].rearrange("p (r c) -> p r c", c=CW)
            def setup_constants():
                zt = pp.tile([P, H], F32)
                nc.vector.memset(zt[:], 0.0)
                nc.sync.dma_start(out=za[:, :H], in_=zt[:])
                nc.sync.dma_start(out=za[:, H:], in_=zt[:, : ZW - H])
                nc.sync.dma_start(out=zb[:, :H], in_=zt[:])
                nc.sync.dma_start(out=zb[:, H:], in_=zt[:, : ZW - H])

                iota64_i = pp.tile([P, CW], I32)
                nc.gpsimd.iota(
                    iota64_i[:], pattern=[[1, CW]], base=0, channel_multiplier=0
                )
                iota64 = pp.tile([P, CW], F32)
                nc.vector.tensor_copy(iota64[:], iota64_i[:])

                ident = pp.tile([P, P], F32)
                make_identity(nc, ident[:])
                # strict-lower-triangle mask for first-occurrence detection
                iov_i = pp.tile([P, 1], I32)
                nc.gpsimd.iota(iov_i[:], pattern=[[1, 1]], base=0, channel_multiplier=1)
                iov = pp.tile([P, 1], F32)
                nc.vector.tensor_copy(iov[:], iov_i[:])
                jT_ps = psp.tile([P, P], F32)
                nc.tensor.transpose(
                    jT_ps[:], iov[:].to_broadcast([P, P]), identity=ident[:]
                )
                ltri = pp.tile([P, P], F32)
                nc.vector.tensor_tensor(
                    ltri[:], iov[:].to_broadcast([P, P]), jT_ps[:], op=ALU.is_gt
                )
                return iota64, ident, ltri

            def phase1(chunk):
                for r in range(chunk * HT, (chunk + 1) * HT):
                    x = xp.tile([P, C], F32)
                    nc.sync.dma_start(
                        out=x[:], in_=xs[:].rearrange("(n p) c -> n p c", p=P)[r]
                    )
                    nc.vector.max(m8_all[:, 8 * r : 8 * r + 8], x[:])
                    nc.vector.max_index(
                        p8_all[:, 8 * r : 8 * r + 8], m8_all[:, 8 * r : 8 * r + 8], x[:]
                    )
                    e = xp.tile([P, C], F32, tag="e")
                    nc.scalar.activation(
                        e[:], x[:], AF.Exp, accum_out=s_all[:, r : r + 1]
                    )
                    nc.gpsimd.indirect_dma_start(
                        out=x_t[:, r : r + 1], out_offset=None,
                        in_=xs_flat,
                        in_offset=bass.IndirectOffsetOnAxis(ap=off[:, r : r + 1], axis=0),
                    )

            def phase2(chunk):
                sl = slice(chunk * HT, (chunk + 1) * HT)
                half = chunk * HT
                # per-sample post pass for this chunk
                nc.scalar.activation(logs[:, sl], s_all[:, sl], AF.Ln)
                nc.vector.tensor_tensor(
                    venc[:, sl], x_t[:, sl], logs[:, sl], op=ALU.subtract
                )
                nc.vector.tensor_scalar_add(venc[:, sl], venc[:, sl], ENC)

                # k' = t*16 + (p>>6); c6 = p & 63
                nc.vector.tensor_copy(
                    pu[:, sl],
                    p8_all[:].rearrange("p (r e) -> p r e", e=8)[:, sl, 0],
                )
                nc.vector.tensor_scalar(
                    hi[:, sl], pu[:, sl], 6, None, op0=ALU.logical_shift_right
                )
                nc.vector.tensor_copy(hi_i[:, sl], hi[:, sl])
                nc.vector.tensor_scalar(
                    c6[:, sl], pu[:, sl], 63, None, op0=ALU.bitwise_and
                )
                nc.vector.tensor_copy(c6f[:, sl], c6[:, sl])
                nc.vector.tensor_scalar(kp[:, sl], t_sb[:, sl], SUB, None, op0=ALU.mult)
                nc.vector.tensor_tensor(kp[:, sl], kp[:, sl], hi_i[:, sl], op=ALU.add)
                nc.vector.tensor_copy(kpf[:, sl], kp[:, sl])

                # payload [P, HT, 64] = (iota64 == c6) * venc
                nc.vector.tensor_tensor(
                    pl3[:, sl], iota64[:, None, :].to_broadcast([P, HT, CW]),
                    c6f[:, sl].to_broadcast([P, HT, CW]), op=ALU.is_equal,
                )
                nc.vector.tensor_tensor(
                    pl3[:, sl], pl3[:, sl], venc[:, sl].to_broadcast([P, HT, CW]),
                    op=ALU.mult,
                )

                # batched duplicate detection: transpose all HT key columns
                # into one PSUM block, then chunk-wide eq / ltri-mask / reduce
                kTb = psp.tile([P, HT * P], F32, tag="kTb")
                kT3 = kTb[:].rearrange("p (r j) -> p r j", j=P)
                eq_all = sp.tile([P, HT * P], F32, tag="eqall")
                eq3 = eq_all[:].rearrange("p (r j) -> p r j", j=P)
                eql_all = sp.tile([P, HT * P], F32, tag="eqlall")
                eql3 = eql_all[:].rearrange("p (r j) -> p r j", j=P)
                prev8 = sp.tile([P, HT], F32, tag="prev8")
                for r in range(half, half + HT):
                    nc.tensor.transpose(
                        kT3[:, r - half, :],
                        kpf[:, r : r + 1].to_broadcast([P, P]),
                        identity=ident[:],
                    )
                nc.vector.tensor_tensor(
                    eq3[:], kpf[:, sl, None].to_broadcast([P, HT, P]), kT3[:],
                    op=ALU.is_equal,
                )
                nc.vector.tensor_tensor(
                    eql3[:], eq3[:], ltri[:, None, :].to_broadcast([P, HT, P]),
                    op=ALU.mult,
                )
                nc.vector.reduce_max(prev8[:], eql3[:], axis=AX.X)
                # keep = (prev8==0); kadj = keep*(kpf-NROW) + NROW
                # (first occurrence -> kpf, duplicate -> garbage row NROW)
                nc.vector.tensor_scalar(
                    keepm[:, sl], prev8[:], 0.0, None, op0=ALU.is_equal
                )
                nc.vector.scalar_tensor_tensor(
                    kadj[:, sl], kpf[:, sl], float(-NROW), keepm[:, sl],
                    op0=ALU.add, op1=ALU.mult,
                )
                nc.vector.tensor_scalar_add(kadj[:, sl], kadj[:, sl], float(NROW))

                # merged payloads via equality matmul
                for r in range(half, half + HT):
                    mg = psp.tile([P, CW], F32, tag="mg")
                    nc.tensor.matmul(
                        mg[:], lhsT=eq3[:, r - half, :], rhs=pl3[:, r, :],
                        start=True, stop=True,
                    )
                    nc.scalar.copy(p23[:, r, :], mg[:])

                kp16 = sp.tile([P, HT], I16, tag="kp16")
                nc.vector.tensor_copy(kp16[:], kadj[:, half : half + HT])
                idx = sp.tile([P, HT * 8], I16, tag="idx")
                idx_v = idx[:].rearrange("p (r e) -> p r e", e=8)
                # trigger on ACT (has slack once this chunk's Exps are done),
                # keeping the SP queue clear for the input stream
                for h in range(8):
                    nc.scalar.dma_start(
                        out=idx_v[0:16, :, h], in_=kp16[16 * h : 16 * h + 16, :]
                    )
                for g in range(1, 8):
                    nc.scalar.dma_start(
                        out=idx[16 * g : 16 * (g + 1), :], in_=idx[0:16, :]
                    )

                for r in range(half, half + HT):
                    dst = bins_a if r % 2 == 0 else bins_b
                    rr = r - half
                    nc.gpsimd.dma_scatter_add(
                        dst[:], p23[:, r : r + 1, :], idx[:, rr * 8 : (rr + 1) * 8],
                        num_idxs=P, num_idxs_reg=P, elem_size=CW,
                        single_packet=SINGLE_PACKET,
                    )

            # software-pipelined main loop: phase2 runs one chunk behind.
            # Constants (incl. 8MB bins zero-init DMA) are emitted after
            # chunk 0's loads so the input stream heads the DMA queues.
            phase1(0)
            iota64, ident, ltri = setup_constants()
            for chunk in range(1, NCH):
                phase1(chunk)
                phase2(chunk - 1)
            phase2(NCH - 1)

            # merge b into a
            for h in range(2):
                sl = slice(h * H, h * H + H)
                ma = pp.tile([P, H], F32, tag="ma")
                mb = pp.tile([P, H], F32, tag="mb")
                nc.sync.dma_start(out=ma[:], in_=za[:, sl])
                nc.sync.dma_start(out=mb[:], in_=zb[:, sl])
                nc.vector.tensor_tensor(ma[:], ma[:], mb[:], op=ALU.add)
                nc.sync.dma_start(out=za[:, sl], in_=ma[:])

            # --- reduce across cores ---
            if n_cores > 1:
                nc.gpsimd.collective_compute(
                    "ReduceScatter", ALU.add,
                    replica_groups=[list(range(n_cores))],
                    ins=[bins_a[0:NROW, :].rearrange("a b -> (a b)")],
                    outs=[rs_out[:].rearrange("a b -> (a b)")],
                )
                shard = rs_out[:]
            else:
                shard = bins_a[0:NROW, :]

            # --- phase 3 ---
            sh_v = shard.rearrange("(t s) c -> t (s c)", s=SUB)
            n_tt = (Tsh + P - 1) // P
            parts = pp.tile([P, n_tt], F32)
            nc.vector.memset(parts[:], 0.0)
            for tt in range(n_tt):
                lo = tt * P
                hi_t = min(Tsh, lo + P)
                npart = hi_t - lo
                S = pp.tile([P, SUB * CW], F32, tag="s3")
                if npart < P:
                    nc.vector.memset(S[:], 0.0)
                nc.sync.dma_start(out=S[:npart], in_=sh_v[lo:hi_t])
                cmt = pp.tile([P, SUB * CW], F32, tag="cm3")
                nc.vector.memset(cmt[:], 0.0)
                nc.sync.dma_start(out=cmt[:npart, 0:C], in_=cmrow[lo:hi_t, :])
                cnt_i = pp.tile([P, SUB * CW], I32, tag="ci3")
                sdiv = pp.tile([P, SUB * CW], F32, tag="sd3")
                nc.vector.tensor_scalar(
                    sdiv[:], S[:], 1.0 / ENC, 0.499, op0=ALU.mult, op1=ALU.add
                )
                nc.vector.tensor_copy(cnt_i[:], sdiv[:])
                cntf = pp.tile([P, SUB * CW], F32, tag="cf3")
                nc.vector.tensor_copy(cntf[:], cnt_i[:])
                encc = pp.tile([P, SUB * CW], F32, tag="ec3")
                nc.vector.tensor_scalar_mul(encc[:], cntf[:], ENC)
                nc.vector.tensor_tensor(S[:], S[:], encc[:], op=ALU.subtract)
                nc.vector.tensor_tensor(cmt[:], cmt[:], cntf[:], op=ALU.add)
                rows = pp.tile([P, 1], F32, tag="rs3")
                nc.vector.reduce_sum(rows[:], cmt[:], axis=AX.X)
                nc.vector.tensor_scalar_max(rows[:], rows[:], 1.0)
                rec = pp.tile([P, 1], F32, tag="rc3")
                nc.vector.reciprocal(rec[:], rows[:])
                nc.vector.tensor_tensor(cmt[:], cmt[:], S[:], op=ALU.mult)
                ws = pp.tile([P, 1], F32, tag="ws3")
                nc.vector.reduce_sum(ws[:], cmt[:], axis=AX.X)
                nc.vector.tensor_tensor(parts[:, tt : tt + 1], ws[:], rec[:], op=ALU.mult)

            # [P] partial sums out; host sums across partitions and cores
            colsum = pp.tile([P, 1], F32)
            nc.vector.reduce_sum(colsum[:], parts[:], axis=AX.X)
            nc.sync.dma_start(out=out[:], in_=colsum[:])

    nc.finalize()
    return nc

_NC_CACHE = None


def kernel(outputs: np.ndarray, targets: np.ndarray, cost_matrix: np.ndarray) -> np.ndarray:
    global _NC_CACHE
    assert outputs.shape == (B, C) and cost_matrix.shape == (C, C)
    if _NC_CACHE is None:
        _NC_CACHE = build_kernel(R, N_CORES)
    nc = _NC_CACHE

    tg32 = np.ascontiguousarray(targets.astype(np.int32))
    xs = np.ascontiguousarray(outputs, dtype=np.float32)
    cmf = np.ascontiguousarray(cost_matrix, dtype=np.float32)
    Tsh = C // N_CORES
    in_maps = [
        {
            "xs": xs[c * R : (c + 1) * R],
            "tg": tg32[c * R : (c + 1) * R],
            "cmrow": cmf[c * Tsh : (c + 1) * Tsh],
        }
        for c in range(N_CORES)
    ]
    res = run_bass_kernel_spmd(nc, in_maps, core_ids=list(range(N_CORES)))
    total = sum(float(res.results[c]["partial"].sum()) for c in range(N_CORES))
    loss = -BETA * total / B
    return np.asarray(loss, dtype=np.float32)


# revision 17
# speedup vs baseline: 1.1830x; 1.1830x over previous
"""Cost-sensitive cross-entropy loss on 8 TRN2 NeuronCores (Bass/Tile).

Data-parallel over the batch: each core streams its 8192x1000 logit shard;
per [128,1000] tile it computes row max (reduce_max), argmax via an
equality-mask * reverse-iota reduce_max, the softmax denominator via the
scalar engine's Exp with fused accumulation, and the target logit via a
per-tile indirect-DMA gather (one offset per partition).  Each sample then
contributes an ENC-encoded one-hot payload (count and value packed in one
f32) which is scatter-added into per-core HBM bins keyed by
(target*16 + predicted>>6).  dma_scatter_add does not serialize
read-modify-writes of the same 256B row within a call, so same-row
duplicates inside each 128-record tile are merged on the tensor engine
(equality-matrix matmul), non-first duplicates are routed to a garbage
row, and the per-tile calls form two WAW-serialized chains into separate
buffers that are then locally summed.  Phase 2 is software-pipelined one
chunk behind phase 1 so dedup/scatter overlap the streaming.  A
ReduceScatter combines bins across cores (class-sharded); each core
decodes counts/values, row-normalizes against the cost matrix and emits
[128] partial loss sums; the host sums them and applies -beta/B.

Self-contained: hardcodes B=65536, C=1000, beta=3.0, 8 cores.
"""
import sys
sys.path.insert(0, '/opt/trn_rl_repo')
import numpy as np
import concourse.bass as bass
import concourse.bacc as bacc
import concourse.mybir as mybir
from concourse import tile
from concourse.masks import make_identity
from concourse.bass_utils import run_bass_kernel_spmd

P = 128
C = 1000
B = 65536
N_CORES = 8
R = B // N_CORES
SUB = 16          # 64-wide sub-rows per class row (16*64 = 1024 >= C)
CW = 64           # scatter row width (f32) = 256B
ENC = 4096.0      # count encoding scale
SINGLE_PACKET = True
BETA = 3.0

F32 = mybir.dt.float32
I32 = mybir.dt.int32
I16 = mybir.dt.int16
U32 = mybir.dt.uint32
AF = mybir.ActivationFunctionType
ALU = mybir.AluOpType
AX = mybir.AxisListType


def build_kernel(R, n_cores, my_rank=None, dma_bufs=6, debug=False):
    NT = R // P
    NROW = C * SUB
    SR = NROW // n_cores
    Tsh = C // n_cores

    nc = bacc.Bacc(None, target_bir_lowering=False)
    xs = nc.dram_tensor("xs", [R, C], F32, kind="ExternalInput")
    tg = nc.dram_tensor("tg", [R], I32, kind="ExternalInput")
    cmrow = nc.dram_tensor("cmrow", [Tsh, C], F32, kind="ExternalInput")
    out = nc.dram_tensor("partial", [P, 1], F32, kind="ExternalOutput")

    bins_a = nc.dram_tensor("bins_a", [NROW + 1, CW], F32)
    bins_b = nc.dram_tensor("bins_b", [NROW + 1, CW], F32)
    rs_out = nc.dram_tensor("rs_out", [SR, CW], F32)

    xs_flat = xs[:].rearrange("a b -> (a b)")[:, None]
    za = bins_a[0:NROW, :].rearrange("(p n) c -> p (n c)", p=P)
    zb = bins_b[0:NROW, :].rearrange("(p n) c -> p (n c)", p=P)
    ZW = za.shape[1]
    H = ZW // 2

    HT = max(1, NT // 8)      # chunk size for pipelined phase 2
    NCH = NT // HT

    with tile.TileContext(nc) as tc:
        with (
            tc.tile_pool(name="xp", bufs=dma_bufs) as xp,
            tc.tile_pool(name="sp", bufs=2) as sp,
            tc.tile_pool(name="pp", bufs=1) as pp,
            tc.tile_pool(name="ps", bufs=2, space="PSUM") as psp,
        ):
            m8_all = pp.tile([P, NT * 8], F32)
            p8_all = pp.tile([P, NT * 8], U32)
            s_all = pp.tile([P, NT], F32)

            # targets + gather offsets
            t_sb = pp.tile([P, NT], I32)
            nc.sync.dma_start(out=t_sb[:], in_=tg[:].rearrange("(n p) -> p n", p=P))
            io_r = pp.tile([P, NT], I32)
            nc.gpsimd.iota(io_r[:], pattern=[[1, NT]], base=0, channel_multiplier=0)
            io_pc = pp.tile([P, 1], I32)
            nc.gpsimd.iota(io_pc[:], pattern=[[1, 1]], base=0, channel_multiplier=C)
            off = pp.tile([P, NT], I32)
            nc.vector.tensor_scalar(off[:], io_r[:], P * C, None, op0=ALU.mult)
            nc.vector.tensor_tensor(off[:], off[:], t_sb[:], op=ALU.add)
            nc.vector.tensor_tensor(
                off[:], off[:], io_pc[:].to_broadcast([P, NT]), op=ALU.add
            )
            x_t = pp.tile([P, NT], F32)

            logs = pp.tile([P, NT], F32)
            venc = pp.tile([P, NT], F32)
            pu = pp.tile([P, NT], U32)
            hi = pp.tile([P, NT], U32)
            hi_i = pp.tile([P, NT], I32)
            c6 = pp.tile([P, NT], U32)
            c6f = pp.tile([P, NT], F32)
            kp = pp.tile([P, NT], I32)
            kpf = pp.tile([P, NT], F32)
            keepm = pp.tile([P, NT], F32)
            payload = pp.tile([P, NT * CW], F32)
            pl3 = payload[:].rearrange("p (r c) -> p r c", c=CW)
            kadj = pp.tile([P, NT], F32)
            pay2 = pp.tile([P, NT * CW], F32)
            p23 = pay2[:].rearrange("p (r c) -> p r c", c=CW)
            def setup_constants():
                zt = pp.tile([P, H], F32)
                nc.vector.memset(zt[:], 0.0)
                nc.sync.dma_start(out=za[:, :H], in_=zt[:])
                nc.sync.dma_start(out=za[:, H:], in_=zt[:, : ZW - H])
                nc.sync.dma_start(out=zb[:, :H], in_=zt[:])
                nc.sync.dma_start(out=zb[:, H:], in_=zt[:, : ZW - H])

                iota64_i = pp.tile([P, CW], I32)
                nc.gpsimd.iota(
                    iota64_i[:], pattern=[[1, CW]], base=0, channel_multiplier=0
                )
                iota64 = pp.tile([P, CW], F32)
                nc.vector.tensor_copy(iota64[:], iota64_i[:])

                ident = pp.tile([P, P], F32)
                make_identity(nc, ident[:])
                # strict-lower-triangle mask for first-occurrence detection
                iov_i = pp.tile([P, 1], I32)
                nc.gpsimd.iota(iov_i[:], pattern=[[1, 1]], base=0, channel_multiplier=1)
                iov = pp.tile([P, 1], F32)
                nc.vector.tensor_copy(iov[:], iov_i[:])
                jT_ps = psp.tile([P, P], F32)
                nc.tensor.transpose(
                    jT_ps[:], iov[:].to_broadcast([P, P]), identity=ident[:]
                )
                ltri = pp.tile([P, P], F32)
                nc.vector.tensor_tensor(
                    ltri[:], iov[:].to_broadcast([P, P]), jT_ps[:], op=ALU.is_gt
                )
                return iota64, ident, ltri

            def phase1(chunk):
                for r in range(chunk * HT, (chunk + 1) * HT):
                    x = xp.tile([P, C], F32)
                    nc.sync.dma_start(
                        out=x[:], in_=xs[:].rearrange("(n p) c -> n p c", p=P)[r]
                    )
                    nc.vector.max(m8_all[:, 8 * r : 8 * r + 8], x[:])
                    nc.vector.max_index(
                        p8_all[:, 8 * r : 8 * r + 8], m8_all[:, 8 * r : 8 * r + 8], x[:]
                    )
                    e = xp.tile([P, C], F32, tag="e")
                    nc.scalar.activation(
                        e[:], x[:], AF.Exp, accum_out=s_all[:, r : r + 1]
                    )
                    nc.gpsimd.indirect_dma_start(
                        out=x_t[:, r : r + 1], out_offset=None,
                        in_=xs_flat,
                        in_offset=bass.IndirectOffsetOnAxis(ap=off[:, r : r + 1], axis=0),
                    )

            def phase2(chunk):
                sl = slice(chunk * HT, (chunk + 1) * HT)
                half = chunk * HT
                # per-sample post pass for this chunk
                nc.scalar.activation(logs[:, sl], s_all[:, sl], AF.Ln)
                nc.vector.tensor_tensor(
                    venc[:, sl], x_t[:, sl], logs[:, sl], op=ALU.subtract
                )
                nc.vector.tensor_scalar_add(venc[:, sl], venc[:, sl], ENC)

                # k' = t*16 + (p>>6); c6 = p & 63
                nc.vector.tensor_copy(
                    pu[:, sl],
                    p8_all[:].rearrange("p (r e) -> p r e", e=8)[:, sl, 0],
                )
                nc.vector.tensor_scalar(
                    hi[:, sl], pu[:, sl], 6, None, op0=ALU.logical_shift_right
                )
                nc.vector.tensor_copy(hi_i[:, sl], hi[:, sl])
                nc.vector.tensor_scalar(
                    c6[:, sl], pu[:, sl], 63, None, op0=ALU.bitwise_and
                )
                nc.vector.tensor_copy(c6f[:, sl], c6[:, sl])
                nc.vector.tensor_scalar(kp[:, sl], t_sb[:, sl], SUB, None, op0=ALU.mult)
                nc.vector.tensor_tensor(kp[:, sl], kp[:, sl], hi_i[:, sl], op=ALU.add)
                nc.vector.tensor_copy(kpf[:, sl], kp[:, sl])

                # payload [P, HT, 64] = (iota64 == c6) * venc
                nc.vector.tensor_tensor(
                    pl3[:, sl], iota64[:, None, :].to_broadcast([P, HT, CW]),
                    c6f[:, sl].to_broadcast([P, HT, CW]), op=ALU.is_equal,
                )
                nc.vector.tensor_tensor(
                    pl3[:, sl], pl3[:, sl], venc[:, sl].to_broadcast([P, HT, CW]),
                    op=ALU.mult,
                )

                # batched duplicate detection: transpose all HT key columns
                # into one PSUM block, then chunk-wide eq / ltri-mask / reduce
                kTb = psp.tile([P, HT * P], F32, tag="kTb")
                kT3 = kTb[:].rearrange("p (r j) -> p r j", j=P)
                eq_all = sp.tile([P, HT * P], F32, tag="eqall")
                eq3 = eq_all[:].rearrange("p (r j) -> p r j", j=P)
                eql_all = sp.tile([P, HT * P], F32, tag="eqlall")
                eql3 = eql_all[:].rearrange("p (r j) -> p r j", j=P)
                prev8 = sp.tile([P, HT], F32, tag="prev8")
                for r in range(half, half + HT):
                    nc.tensor.transpose(
                        kT3[:, r - half, :],
                        kpf[:, r : r + 1].to_broadcast([P, P]),
                        identity=ident[:],
                    )
                nc.vector.tensor_tensor(
                    eq3[:], kpf[:, sl, None].to_broadcast([P, HT, P]), kT3[:],
                    op=ALU.is_equal,
                )
                nc.vector.tensor_tensor(
                    eql3[:], eq3[:], ltri[:, None, :].to_broadcast([P, HT, P]),
                    op=ALU.mult,
                )
                nc.vector.reduce_max(prev8[:], eql3[:], axis=AX.X)
                # keep = (prev8==0); kadj = keep*(kpf-NROW) + NROW
                # (first occurrence -> kpf, duplicate -> garbage row NROW)
                nc.vector.tensor_scalar(
                    keepm[:, sl], prev8[:], 0.0, None, op0=ALU.is_equal
                )
                nc.vector.scalar_tensor_tensor(
                    kadj[:, sl], kpf[:, sl], float(-NROW), keepm[:, sl],
                    op0=ALU.add, op1=ALU.mult,
                )
                nc.vector.tensor_scalar_add(kadj[:, sl], kadj[:, sl], float(NROW))

                # merged payloads via equality matmul
                for r in range(half, half + HT):
                    mg = psp.tile([P, CW], F32, tag="mg")
                    nc.tensor.matmul(
                        mg[:], lhsT=eq3[:, r - half, :], rhs=pl3[:, r, :],
                        start=True, stop=True,
                    )
                    nc.scalar.copy(p23[:, r, :], mg[:])

                kp16 = sp.tile([P, HT], I16, tag="kp16")
                nc.vector.tensor_copy(kp16[:], kadj[:, half : half + HT])
                idx = sp.tile([P, HT * 8], I16, tag="idx")
                idx_v = idx[:].rearrange("p (r e) -> p r e", e=8)
                for h in range(8):
                    nc.sync.dma_start(
                        out=idx_v[0:16, :, h], in_=kp16[16 * h : 16 * h + 16, :]
                    )
                for g in range(1, 8):
                    nc.sync.dma_start(
                        out=idx[16 * g : 16 * (g + 1), :], in_=idx[0:16, :]
                    )

                for r in range(half, half + HT):
                    dst = bins_a if r % 2 == 0 else bins_b
                    rr = r - half
                    nc.gpsimd.dma_scatter_add(
                        dst[:], p23[:, r : r + 1, :], idx[:, rr * 8 : (rr + 1) * 8],
                        num_idxs=P, num_idxs_reg=P, elem_size=CW,
                        single_packet=SINGLE_PACKET,
                    )

            # software-pipelined main loop: phase2 runs one chunk behind.
            # Constants (incl. 8MB bins zero-init DMA) are emitted after
            # chunk 0's loads so the input stream heads the DMA queues.
            phase1(0)
            iota64, ident, ltri = setup_constants()
            for chunk in range(1, NCH):
                phase1(chunk)
                phase2(chunk - 1)
            phase2(NCH - 1)

            # merge b into a
            for h in range(2):
                sl = slice(h * H, h * H + H)
                ma = pp.tile([P, H], F32, tag="ma")
                mb = pp.tile([P, H], F32, tag="mb")
                nc.sync.dma_start(out=ma[:], in_=za[:, sl])
                nc.sync.dma_start(out=mb[:], in_=zb[:, sl])
                nc.vector.tensor_tensor(ma[:], ma[:], mb[:], op=ALU.add)
                nc.sync.dma_start(out=za[:, sl], in_=ma[:])

            # --- reduce across cores ---
            if n_cores > 1:
                nc.gpsimd.collective_compute(
                    "ReduceScatter", ALU.add,
                    replica_groups=[list(range(n_cores))],
                    ins=[bins_a[0:NROW, :].rearrange("a b -> (a b)")],
                    outs=[rs_out[:].rearrange("a b -> (a b)")],
                )
                shard = rs_out[:]
            else:
                shard = bins_a[0:NROW, :]

            # --- phase 3 ---
            sh_v = shard.rearrange("(t s) c -> t (s c)", s=SUB)
            n_tt = (Tsh + P - 1) // P
            parts = pp.tile([P, n_tt], F32)
            nc.vector.memset(parts[:], 0.0)
            for tt in range(n_tt):
                lo = tt * P
                hi_t = min(Tsh, lo + P)
                npart = hi_t - lo
                S = pp.tile([P, SUB * CW], F32, tag="s3")
                if npart < P:
                    nc.vector.memset(S[:], 0.0)
                nc.sync.dma_start(out=S[:npart], in_=sh_v[lo:hi_t])
                cmt = pp.tile([P, SUB * CW], F32, tag="cm3")
                nc.vector.memset(cmt[:], 0.0)
                nc.sync.dma_start(out=cmt[:npart, 0:C], in_=cmrow[lo:hi_t, :])
                cnt_i = pp.tile([P, SUB * CW], I32, tag="ci3")
                sdiv = pp.tile([P, SUB * CW], F32, tag="sd3")
                nc.vector.tensor_scalar(
                    sdiv[:], S[:], 1.0 / ENC, 0.499, op0=ALU.mult, op1=ALU.add
                )
                nc.vector.tensor_copy(cnt_i[:], sdiv[:])
                cntf = pp.tile([P, SUB * CW], F32, tag="cf3")
                nc.vector.tensor_copy(cntf[:], cnt_i[:])
                encc = pp.tile([P, SUB * CW], F32, tag="ec3")
                nc.vector.tensor_scalar_mul(encc[:], cntf[:], ENC)
                nc.vector.tensor_tensor(S[:], S[:], encc[:], op=ALU.subtract)
                nc.vector.tensor_tensor(cmt[:], cmt[:], cntf[:], op=ALU.add)
                rows = pp.tile([P, 1], F32, tag="rs3")
                nc.vector.reduce_sum(rows[:], cmt[:], axis=AX.X)
                nc.vector.tensor_scalar_max(rows[:], rows[:], 1.0)
                rec = pp.tile([P, 1], F32, tag="rc3")
                nc.vector.reciprocal(rec[:], rows[:])
                nc.vector.tensor_tensor(cmt[:], cmt[:], S[:], op=ALU.mult)
                ws = pp.tile([P, 1], F32, tag="ws3")
                nc.vector.reduce_sum(ws[:], cmt[:], axis=AX.X)
                nc.vector.tensor_tensor(parts[:, tt : tt + 1], ws[:], rec[:], op=ALU.mult)

            # [P] partial sums out; host sums across partitions and cores
            colsum = pp.tile([P, 1], F32)
            nc.vector.reduce_sum(colsum[:], parts[:], axis=AX.X)
            nc.sync.dma_start(out=out[:], in_=colsum[:])

    nc.finalize()
    return nc

_NC_CACHE = None


def kernel(outputs: np.ndarray, targets: np.ndarray, cost_matrix: np.ndarray) -> np.ndarray:
    global _NC_CACHE
    assert outputs.shape == (B, C) and cost_matrix.shape == (C, C)
    if _NC_CACHE is None:
        _NC_CACHE = build_kernel(R, N_CORES)
    nc = _NC_CACHE

    tg32 = np.ascontiguousarray(targets.astype(np.int32))
    xs = np.ascontiguousarray(outputs, dtype=np.float32)
    cmf = np.ascontiguousarray(cost_matrix, dtype=np.float32)
    Tsh = C // N_CORES
    in_maps = [
        {
            "xs": xs[c * R : (c + 1) * R],
            "tg": tg32[c * R : (c + 1) * R],
            "cmrow": cmf[c * Tsh : (c + 1) * Tsh],
        }
        for c in range(N_CORES)
    ]
    res = run_bass_kernel_spmd(nc, in_maps, core_ids=list(range(N_CORES)))
    total = sum(float(res.results[c]["partial"].sum()) for c in range(N_CORES))
    loss = -BETA * total / B
    return np.asarray(loss, dtype=np.float32)
